# revision 17
# baseline (speedup 1.0000x reference)
"""Dcls3d (learnable-position dilated conv3d) Trainium2 kernel.

Reference computes:
  K = trilinear-scatter(weight, P) -> (64, 32, 5, 5, 5)
  out = conv3d(x, K, stride 1, pad 2) + bias     x: (2,32,16,32,32) -> out: (2,64,16,32,32)

Strategy (8 cores): shard (batch n in {0,1}) x (4 chunks of 4 output d-planes).
Each core runs an implicit-GEMM direct conv:
  - input slab (zero-padded on host) replicated 4x in SBUF, w-shifted by
    delta=0..3, giving a 128-partition (delta, ic) contraction axis.
  - for each of 25 (l, j) kernel-tap pairs: one matmul contracting
    (4 w-taps x 32 ic) = 128, M=64 out-channels, N=512 outputs, accumulating
    in PSUM; the i=4 leftover tap runs as a K=32 matmul off the delta-group.
  - bias added during PSUM->SBUF copyback; one 1MB store per core.
"""

import numpy as np

import concourse.bass as bass
import concourse.bacc as bacc
import concourse.mybir as mybir
from concourse.bass_utils import run_bass_kernel_spmd
from concourse.tile import TileContext

# ---- problem constants (hardcoded per contract) ----
N, IC, D, H, W = 2, 32, 16, 32, 32
OC = 64
KC = 16
PAD = 2
DP, HP, WP = D + 2 * PAD, H + 2 * PAD, W + 2 * PAD  # 20, 36, 36
DCHUNK = 4              # output d-planes per core
DSLAB = DCHUNK + 4      # input d-planes per core (halo 2 each side)
SLABF = DSLAB * HP * WP  # 8*36*36 = 10368
XS_COLS = SLABF + 4     # slack so the delta-shifted loads stay in bounds
NTAPS_LJ = 25
OUTF = DCHUNK * H * W   # 4096 outputs per (core, oc)

_NC_CACHE = {}


def _construct_K(weight, P):
    """Exact numpy port of reference.construct_kernel for ks=(5,5,5)."""
    Pp = P + np.float32(2.0)
    Pf = np.floor(Pp)
    R = Pp - Pf
    P1, P2, P3 = Pf[0], Pf[1], Pf[2]
    R1, R2, R3 = R[0], R[1], R[2]
    g = np.arange(5, dtype=P.dtype)[:, None, None, None]
    aL = (g == P1) * (1.0 - R1) + (g == P1 + 1.0) * R1
    aJ = (g == P3) * (1.0 - R3) + (g == P3 + 1.0) * R3
    aI = (g == P2) * (1.0 - R2) + (g == P2 + 1.0) * R2
    K = np.einsum("ock,lock,jock,iock->oclji", weight, aL, aJ, aI, optimize=True)
    return np.ascontiguousarray(K.astype(np.float32))


LJ_A = [lj for lj in range(NTAPS_LJ) if lj % 2 == 0]  # col-group 0 taps
LJ_B = [lj for lj in range(NTAPS_LJ) if lj % 2 == 1]  # col-group 1 taps
ROW_PACK = False  # leftover i=4 taps spread across PE row groups


def _build_nc_packed(mm="bf16"):
    """v1: col-group packed (2 taps concurrently on PE) + row-packed i=4."""
    key = ("v1", mm, ROW_PACK)
    if key in _NC_CACHE:
        return _NC_CACHE[key]
    f32 = mybir.dt.float32
    mdt = {"f32": f32, "bf16": mybir.dt.bfloat16}[mm]
    nc = bacc.Bacc()
    xs = nc.dram_tensor("xs", [IC, XS_COLS], mdt, kind="ExternalInput")
    kta = nc.dram_tensor("kta", [128, len(LJ_A) * OC], mdt, kind="ExternalInput")
    ktb = nc.dram_tensor("ktb", [128, len(LJ_B) * OC], mdt, kind="ExternalInput")
    kt4 = nc.dram_tensor("kt4", [128, NTAPS_LJ * OC], mdt, kind="ExternalInput")
    bias = nc.dram_tensor("bias", [OC, 1], f32, kind="ExternalInput")
    out = nc.dram_tensor("out", [OC, OUTF], f32, kind="ExternalOutput")

    with TileContext(nc) as tc:
        with (
            tc.tile_pool(name="const", bufs=1) as cpool,
            tc.tile_pool(name="psum", bufs=4, space="PSUM") as ppool,
        ):
            kta_sb = cpool.tile([128, len(LJ_A) * OC], mdt)
            nc.sync.dma_start(out=kta_sb, in_=kta[:, :])
            ktb_sb = cpool.tile([128, len(LJ_B) * OC], mdt)
            nc.sync.dma_start(out=ktb_sb, in_=ktb[:, :])
            kt4_sb = cpool.tile([128, NTAPS_LJ * OC], mdt)
            nc.sync.dma_start(out=kt4_sb, in_=kt4[:, :])
            bias_sb = cpool.tile([OC, 1], f32)
            nc.sync.dma_start(out=bias_sb, in_=bias[:, :])
            xrep = cpool.tile([128, SLABF], mdt)
            for dl in range(4):
                nc.sync.dma_start(
                    out=xrep[dl * IC : (dl + 1) * IC, :], in_=xs[:, dl : dl + SLABF]
                )
            obuf = cpool.tile([OC, OUTF], f32)

            xrep_r = xrep.rearrange("p (r w) -> p r w", w=WP)

            for t in range(8):
                d, h0 = divmod(t, 2)
                h0 *= 16
                ps = ppool.tile([128, 512], f32)
                rows = {}
                for lj in range(NTAPS_LJ):
                    l, j = divmod(lj, 5)
                    rows[lj] = (d + l) * HP + h0 + j
                # big taps: A into psum[0:64] (cols 0-63), B into psum[64:128]
                for s in range(len(LJ_A)):
                    for grp, ljs, ktsb in ((0, LJ_A, kta_sb), (1, LJ_B, ktb_sb)):
                        if s >= len(ljs):
                            continue
                        lj = ljs[s]
                        r = rows[lj]
                        nc.tensor.matmul(
                            ps[grp * 64 : grp * 64 + 64, :],
                            ktsb[:, s * OC : (s + 1) * OC],
                            xrep_r[:, r : r + 16, 0:W],
                            start=(s == 0),
                            stop=False,
                            skip_group_check=True,
                            tile_position=(0, grp * 64),
                        )
                # i=4 leftovers: col group = lj % 2, row group from ROW_PACK
                for lj in range(NTAPS_LJ):
                    dlt = (lj % 4) if ROW_PACK else 0
                    grp = lj % 2
                    r = rows[lj]
                    last = lj >= NTAPS_LJ - 2  # lj 23 (grp1) and 24 (grp0)
                    nc.tensor.matmul(
                        ps[grp * 64 : grp * 64 + 64, :],
                        kt4_sb[32 * dlt : 32 * (dlt + 1), lj * OC : (lj + 1) * OC],
                        xrep_r[32 * dlt : 32 * (dlt + 1), r : r + 16, 4 - dlt : 4 - dlt + W],
                        start=False,
                        stop=last,
                        skip_group_check=True,
                        tile_position=(32 * dlt, grp * 64),
                    )
                oslice = obuf[:, t * 512 : (t + 1) * 512]
                nc.vector.tensor_scalar_add(out=oslice, in0=ps[0:64, :], scalar1=bias_sb)
                nc.vector.tensor_tensor(
                    out=oslice, in0=ps[64:128, :], in1=oslice,
                    op=mybir.AluOpType.add,
                )
            nc.sync.dma_start(out=out[:, :], in_=obuf)
    nc.finalize()
    _NC_CACHE[key] = nc
    return nc


def _build_nc(mm="bf16"):
    key = ("v0", mm)
    if key in _NC_CACHE:
        return _NC_CACHE[key]
    f32 = mybir.dt.float32
    mdt = {"f32": f32, "bf16": mybir.dt.bfloat16}[mm]
    nc = bacc.Bacc()
    xs = nc.dram_tensor("xs", [IC, XS_COLS], mdt, kind="ExternalInput")
    kt = nc.dram_tensor("kt", [128, NTAPS_LJ * OC], mdt, kind="ExternalInput")
    kt4 = nc.dram_tensor("kt4", [IC, NTAPS_LJ * OC], mdt, kind="ExternalInput")
    bias = nc.dram_tensor("bias", [OC, 1], f32, kind="ExternalInput")
    out = nc.dram_tensor("out", [OC, OUTF], f32, kind="ExternalOutput")

    with TileContext(nc) as tc:
        with (
            tc.tile_pool(name="const", bufs=1) as cpool,
            tc.tile_pool(name="psum", bufs=4, space="PSUM") as ppool,
        ):
            xrep = cpool.tile([128, SLABF], mdt)
            # partition p = dl*32+ic holds xs[ic, dl : dl+SLABF] (w-shift by dl)
            for dl in range(4):
                nc.sync.dma_start(
                    out=xrep[dl * IC : (dl + 1) * IC, :], in_=xs[:, dl : dl + SLABF]
                )
            kt_sb = cpool.tile([128, NTAPS_LJ * OC], mdt)
            nc.sync.dma_start(out=kt_sb, in_=kt[:, :])
            kt4_sb = cpool.tile([IC, NTAPS_LJ * OC], mdt)
            nc.sync.dma_start(out=kt4_sb, in_=kt4[:, :])
            bias_sb = cpool.tile([OC, 1], f32)
            nc.sync.dma_start(out=bias_sb, in_=bias[:, :])
            obuf = cpool.tile([OC, OUTF], f32)

            # view xrep free dim as (row, w) where row = d*HP + h
            xrep_r = xrep.rearrange("p (r w) -> p r w", w=WP)

            for t in range(8):  # out tile: 512 outputs = 16 h-rows x 32 w
                d, h0 = divmod(t, 2)
                h0 *= 16
                ps = ppool.tile([OC, 512], f32)
                for lj in range(NTAPS_LJ):
                    l, j = divmod(lj, 5)
                    r = (d + l) * HP + h0 + j
                    rhs = xrep_r[:, r : r + 16, 0:W]
                    nc.tensor.matmul(
                        ps,
                        kt_sb[:, lj * OC : (lj + 1) * OC],
                        rhs,
                        start=(lj == 0),
                        stop=False,
                    )
                    rhs4 = xrep_r[0:IC, r : r + 16, 4 : 4 + W]
                    nc.tensor.matmul(
                        ps,
                        kt4_sb[:, lj * OC : (lj + 1) * OC],
                        rhs4,
                        start=False,
                        stop=(lj == NTAPS_LJ - 1),
                    )
                nc.vector.tensor_scalar_add(
                    out=obuf[:, t * 512 : (t + 1) * 512], in0=ps, scalar1=bias_sb
                )
            nc.sync.dma_start(out=out[:, :], in_=obuf)
    nc.finalize()
    _NC_CACHE[key] = nc
    return nc


def kernel(x, weight, P, bias, mm="bf16", ver="v1"):
    import ml_dtypes

    x = np.ascontiguousarray(np.asarray(x, dtype=np.float32))
    weight = np.asarray(weight, dtype=np.float32)
    P = np.asarray(P, dtype=np.float32)
    bias = np.asarray(bias, dtype=np.float32)
    mnp = {"f32": np.float32, "bf16": ml_dtypes.bfloat16}[mm]

    K = _construct_K(weight, P)  # (oc, ic, l, j, i)
    # lhsT layouts: partition=(i, ic), free=(l*5+j slot, oc)
    Kt = K.transpose(4, 1, 2, 3, 0)  # (i, ic, l, j, oc)
    KtF = Kt.reshape(5, IC, NTAPS_LJ, OC)
    bias_in = np.ascontiguousarray(bias.reshape(OC, 1))

    xpad = np.pad(x, ((0, 0), (0, 0), (PAD, PAD), (PAD, PAD), (PAD, PAD)))

    if ver == "v0":
        kt = np.ascontiguousarray(KtF[:4].reshape(128, NTAPS_LJ * OC).astype(mnp))
        kt4 = np.ascontiguousarray(KtF[4].reshape(IC, NTAPS_LJ * OC).astype(mnp))
        extra = {"kt": kt, "kt4": kt4}
        build = _build_nc
    else:
        kta = np.ascontiguousarray(
            KtF[:4][:, :, LJ_A, :].reshape(128, len(LJ_A) * OC).astype(mnp)
        )
        ktb = np.ascontiguousarray(
            KtF[:4][:, :, LJ_B, :].reshape(128, len(LJ_B) * OC).astype(mnp)
        )
        kt4 = np.zeros((128, NTAPS_LJ * OC), mnp)
        for lj in range(NTAPS_LJ):
            dlt = (lj % 4) if ROW_PACK else 0
            kt4[32 * dlt : 32 * (dlt + 1), lj * OC : (lj + 1) * OC] = KtF[
                4, :, lj, :
            ].astype(mnp)
        extra = {"kta": kta, "ktb": ktb, "kt4": kt4}
        build = _build_nc_packed

    in_maps = []
    for ci in range(8):
        n, dc = divmod(ci, 4)
        slab = xpad[n, :, 4 * dc : 4 * dc + DSLAB].reshape(IC, SLABF)
        xs = np.zeros((IC, XS_COLS), mnp)
        xs[:, :SLABF] = slab.astype(mnp)
        in_maps.append({"xs": xs, "bias": bias_in, **extra})

    global _last_in_maps, _last_mm, _last_build
    _last_in_maps = in_maps
    _last_mm = mm
    _last_build = build
    nc = build(mm)
    res = run_bass_kernel_spmd(nc, in_maps, core_ids=list(range(8)))

    out = np.empty((N, OC, D, H, W), np.float32)
    for ci in range(8):
        n, dc = divmod(ci, 4)
        out[n, :, 4 * dc : 4 * dc + DCHUNK] = res.results[ci]["out"].reshape(
            OC, DCHUNK, H, W
        )
    return out


# revision 19
# speedup vs baseline: 1.0364x; 1.0364x over previous
"""Dcls3d (learnable-position dilated conv3d) Trainium2 kernel.

Reference computes:
  K = trilinear-scatter(weight, P) -> (64, 32, 5, 5, 5)
  out = conv3d(x, K, stride 1, pad 2) + bias     x: (2,32,16,32,32) -> out: (2,64,16,32,32)

Strategy (8 cores): shard (batch n in {0,1}) x (4 chunks of 4 output d-planes).
Each core runs an implicit-GEMM direct conv:
  - input slab (zero-padded on host) replicated 4x in SBUF, w-shifted by
    delta=0..3, giving a 128-partition (delta, ic) contraction axis.
  - for each of 25 (l, j) kernel-tap pairs: one matmul contracting
    (4 w-taps x 32 ic) = 128, M=64 out-channels, N=512 outputs, accumulating
    in PSUM; the i=4 leftover tap runs as a K=32 matmul off the delta-group.
  - bias added during PSUM->SBUF copyback; one 1MB store per core.
"""

import numpy as np

import concourse.bass as bass
import concourse.bacc as bacc
import concourse.mybir as mybir
from concourse.bass_utils import run_bass_kernel_spmd
from concourse.tile import TileContext

# ---- problem constants (hardcoded per contract) ----
N, IC, D, H, W = 2, 32, 16, 32, 32
OC = 64
KC = 16
PAD = 2
DP, HP, WP = D + 2 * PAD, H + 2 * PAD, W + 2 * PAD  # 20, 36, 36
DCHUNK = 4              # output d-planes per core
DSLAB = DCHUNK + 4      # input d-planes per core (halo 2 each side)
SLABF = DSLAB * HP * WP  # 8*36*36 = 10368
XS_COLS = SLABF + 4     # slack so the delta-shifted loads stay in bounds
NTAPS_LJ = 25
OUTF = DCHUNK * H * W   # 4096 outputs per (core, oc)

_NC_CACHE = {}


def _construct_K(weight, P):
    """Exact numpy port of reference.construct_kernel for ks=(5,5,5)."""
    Pp = P + np.float32(2.0)
    Pf = np.floor(Pp)
    R = Pp - Pf
    P1, P2, P3 = Pf[0], Pf[1], Pf[2]
    R1, R2, R3 = R[0], R[1], R[2]
    g = np.arange(5, dtype=P.dtype)[:, None, None, None]
    aL = (g == P1) * (1.0 - R1) + (g == P1 + 1.0) * R1
    aJ = (g == P3) * (1.0 - R3) + (g == P3 + 1.0) * R3
    aI = (g == P2) * (1.0 - R2) + (g == P2 + 1.0) * R2
    K = np.einsum("ock,lock,jock,iock->oclji", weight, aL, aJ, aI, optimize=True)
    return np.ascontiguousarray(K.astype(np.float32))


LJ_A = [lj for lj in range(NTAPS_LJ) if lj % 2 == 0]  # col-group 0 taps
LJ_B = [lj for lj in range(NTAPS_LJ) if lj % 2 == 1]  # col-group 1 taps
ROW_PACK = False  # leftover i=4 taps spread across PE row groups


def _build_nc_packed(mm="bf16"):
    """v1: col-group packed (2 taps concurrently on PE) + row-packed i=4."""
    key = ("v1", mm, ROW_PACK)
    if key in _NC_CACHE:
        return _NC_CACHE[key]
    f32 = mybir.dt.float32
    mdt = {"f32": f32, "bf16": mybir.dt.bfloat16}[mm]
    nc = bacc.Bacc()
    xs = nc.dram_tensor("xs", [IC, XS_COLS], mdt, kind="ExternalInput")
    kta = nc.dram_tensor("kta", [128, len(LJ_A) * OC], mdt, kind="ExternalInput")
    ktb = nc.dram_tensor("ktb", [128, len(LJ_B) * OC], mdt, kind="ExternalInput")
    kt4 = nc.dram_tensor("kt4", [128, NTAPS_LJ * OC], mdt, kind="ExternalInput")
    bias = nc.dram_tensor("bias", [OC, 1], f32, kind="ExternalInput")
    out = nc.dram_tensor("out", [OC, OUTF], f32, kind="ExternalOutput")

    HALF = 6 * HP * WP  # six d-planes per xrep half
    with TileContext(nc) as tc:
        with (
            tc.tile_pool(name="const", bufs=1) as cpool,
            tc.tile_pool(name="psum", bufs=8, space="PSUM") as ppool,
        ):
            kta_sb = cpool.tile([128, len(LJ_A) * OC], mdt)
            nc.sync.dma_start(out=kta_sb, in_=kta[:, :])
            ktb_sb = cpool.tile([128, len(LJ_B) * OC], mdt)
            nc.sync.dma_start(out=ktb_sb, in_=ktb[:, :])
            kt4_sb = cpool.tile([128, NTAPS_LJ * OC], mdt)
            nc.sync.dma_start(out=kt4_sb, in_=kt4[:, :])
            bias_sb = cpool.tile([OC, 1], f32)
            nc.sync.dma_start(out=bias_sb, in_=bias[:, :])
            # input slab split in two halves (planes 0-5 / 2-7) so out d=0,1
            # compute starts while the second half still loads
            xrepA = cpool.tile([128, HALF], mdt)
            xrepB = cpool.tile([128, HALF], mdt)
            for dl in range(4):
                nc.sync.dma_start(
                    out=xrepA[dl * IC : (dl + 1) * IC, :], in_=xs[:, dl : dl + HALF]
                )
            for dl in range(4):
                nc.sync.dma_start(
                    out=xrepB[dl * IC : (dl + 1) * IC, :],
                    in_=xs[:, 2 * HP * WP + dl : 2 * HP * WP + dl + HALF],
                )
            obufs = [cpool.tile([OC, H * W], f32, name=f"obuf{d}") for d in range(4)]

            xrepA_r = xrepA.rearrange("p (r w) -> p r w", w=WP)
            xrepB_r = xrepB.rearrange("p (r w) -> p r w", w=WP)

            for t in range(8):
                d, h0 = divmod(t, 2)
                h0 *= 16
                xrep_r = xrepA_r if d < 2 else xrepB_r
                dbase = 0 if d < 2 else 2  # plane offset of the half
                ps = ppool.tile([128, 512], f32)
                rows = {}
                for lj in range(NTAPS_LJ):
                    l, j = divmod(lj, 5)
                    rows[lj] = (d + l - dbase) * HP + h0 + j
                # big taps: A into psum[0:64] (cols 0-63), B into psum[64:128]
                for s in range(len(LJ_A)):
                    for grp, ljs, ktsb in ((0, LJ_A, kta_sb), (1, LJ_B, ktb_sb)):
                        if s >= len(ljs):
                            continue
                        lj = ljs[s]
                        r = rows[lj]
                        nc.tensor.matmul(
                            ps[grp * 64 : grp * 64 + 64, :],
                            ktsb[:, s * OC : (s + 1) * OC],
                            xrep_r[:, r : r + 16, 0:W],
                            start=(s == 0),
                            stop=False,
                            skip_group_check=True,
                            tile_position=(0, grp * 64),
                        )
                # i=4 leftovers: col group = lj % 2, row group from ROW_PACK
                for lj in range(NTAPS_LJ):
                    dlt = (lj % 4) if ROW_PACK else 0
                    grp = lj % 2
                    r = rows[lj]
                    last = lj >= NTAPS_LJ - 2  # lj 23 (grp1) and 24 (grp0)
                    nc.tensor.matmul(
                        ps[grp * 64 : grp * 64 + 64, :],
                        kt4_sb[32 * dlt : 32 * (dlt + 1), lj * OC : (lj + 1) * OC],
                        xrep_r[32 * dlt : 32 * (dlt + 1), r : r + 16, 4 - dlt : 4 - dlt + W],
                        start=False,
                        stop=last,
                        skip_group_check=True,
                        tile_position=(32 * dlt, grp * 64),
                    )
                oslice = obufs[d][:, (t % 2) * 512 : (t % 2) * 512 + 512]
                nc.vector.tensor_scalar_add(out=oslice, in0=ps[0:64, :], scalar1=bias_sb)
                nc.vector.tensor_tensor(
                    out=oslice, in0=ps[64:128, :], in1=oslice,
                    op=mybir.AluOpType.add,
                )
                if t % 2 == 1:  # both h-halves of plane d done -> store it
                    nc.sync.dma_start(
                        out=out[:, d * H * W : (d + 1) * H * W], in_=obufs[d]
                    )
    nc.finalize()
    _NC_CACHE[key] = nc
    return nc


def _build_nc(mm="bf16"):
    key = ("v0", mm)
    if key in _NC_CACHE:
        return _NC_CACHE[key]
    f32 = mybir.dt.float32
    mdt = {"f32": f32, "bf16": mybir.dt.bfloat16}[mm]
    nc = bacc.Bacc()
    xs = nc.dram_tensor("xs", [IC, XS_COLS], mdt, kind="ExternalInput")
    kt = nc.dram_tensor("kt", [128, NTAPS_LJ * OC], mdt, kind="ExternalInput")
    kt4 = nc.dram_tensor("kt4", [IC, NTAPS_LJ * OC], mdt, kind="ExternalInput")
    bias = nc.dram_tensor("bias", [OC, 1], f32, kind="ExternalInput")
    out = nc.dram_tensor("out", [OC, OUTF], f32, kind="ExternalOutput")

    with TileContext(nc) as tc:
        with (
            tc.tile_pool(name="const", bufs=1) as cpool,
            tc.tile_pool(name="psum", bufs=4, space="PSUM") as ppool,
        ):
            xrep = cpool.tile([128, SLABF], mdt)
            # partition p = dl*32+ic holds xs[ic, dl : dl+SLABF] (w-shift by dl)
            for dl in range(4):
                nc.sync.dma_start(
                    out=xrep[dl * IC : (dl + 1) * IC, :], in_=xs[:, dl : dl + SLABF]
                )
            kt_sb = cpool.tile([128, NTAPS_LJ * OC], mdt)
            nc.sync.dma_start(out=kt_sb, in_=kt[:, :])
            kt4_sb = cpool.tile([IC, NTAPS_LJ * OC], mdt)
            nc.sync.dma_start(out=kt4_sb, in_=kt4[:, :])
            bias_sb = cpool.tile([OC, 1], f32)
            nc.sync.dma_start(out=bias_sb, in_=bias[:, :])
            obuf = cpool.tile([OC, OUTF], f32)

            # view xrep free dim as (row, w) where row = d*HP + h
            xrep_r = xrep.rearrange("p (r w) -> p r w", w=WP)

            for t in range(8):  # out tile: 512 outputs = 16 h-rows x 32 w
                d, h0 = divmod(t, 2)
                h0 *= 16
                ps = ppool.tile([OC, 512], f32)
                for lj in range(NTAPS_LJ):
                    l, j = divmod(lj, 5)
                    r = (d + l) * HP + h0 + j
                    rhs = xrep_r[:, r : r + 16, 0:W]
                    nc.tensor.matmul(
                        ps,
                        kt_sb[:, lj * OC : (lj + 1) * OC],
                        rhs,
                        start=(lj == 0),
                        stop=False,
                    )
                    rhs4 = xrep_r[0:IC, r : r + 16, 4 : 4 + W]
                    nc.tensor.matmul(
                        ps,
                        kt4_sb[:, lj * OC : (lj + 1) * OC],
                        rhs4,
                        start=False,
                        stop=(lj == NTAPS_LJ - 1),
                    )
                nc.vector.tensor_scalar_add(
                    out=obuf[:, t * 512 : (t + 1) * 512], in0=ps, scalar1=bias_sb
                )
            nc.sync.dma_start(out=out[:, :], in_=obuf)
    nc.finalize()
    _NC_CACHE[key] = nc
    return nc


def kernel(x, weight, P, bias, mm="bf16", ver="v1"):
    import ml_dtypes

    x = np.ascontiguousarray(np.asarray(x, dtype=np.float32))
    weight = np.asarray(weight, dtype=np.float32)
    P = np.asarray(P, dtype=np.float32)
    bias = np.asarray(bias, dtype=np.float32)
    mnp = {"f32": np.float32, "bf16": ml_dtypes.bfloat16}[mm]

    K = _construct_K(weight, P)  # (oc, ic, l, j, i)
    # lhsT layouts: partition=(i, ic), free=(l*5+j slot, oc)
    Kt = K.transpose(4, 1, 2, 3, 0)  # (i, ic, l, j, oc)
    KtF = Kt.reshape(5, IC, NTAPS_LJ, OC)
    bias_in = np.ascontiguousarray(bias.reshape(OC, 1))

    xpad = np.pad(x, ((0, 0), (0, 0), (PAD, PAD), (PAD, PAD), (PAD, PAD)))

    if ver == "v0":
        kt = np.ascontiguousarray(KtF[:4].reshape(128, NTAPS_LJ * OC).astype(mnp))
        kt4 = np.ascontiguousarray(KtF[4].reshape(IC, NTAPS_LJ * OC).astype(mnp))
        extra = {"kt": kt, "kt4": kt4}
        build = _build_nc
    else:
        kta = np.ascontiguousarray(
            KtF[:4][:, :, LJ_A, :].reshape(128, len(LJ_A) * OC).astype(mnp)
        )
        ktb = np.ascontiguousarray(
            KtF[:4][:, :, LJ_B, :].reshape(128, len(LJ_B) * OC).astype(mnp)
        )
        kt4 = np.zeros((128, NTAPS_LJ * OC), mnp)
        for lj in range(NTAPS_LJ):
            dlt = (lj % 4) if ROW_PACK else 0
            kt4[32 * dlt : 32 * (dlt + 1), lj * OC : (lj + 1) * OC] = KtF[
                4, :, lj, :
            ].astype(mnp)
        extra = {"kta": kta, "ktb": ktb, "kt4": kt4}
        build = _build_nc_packed

    in_maps = []
    for ci in range(8):
        n, dc = divmod(ci, 4)
        slab = xpad[n, :, 4 * dc : 4 * dc + DSLAB].reshape(IC, SLABF)
        xs = np.zeros((IC, XS_COLS), mnp)
        xs[:, :SLABF] = slab.astype(mnp)
        in_maps.append({"xs": xs, "bias": bias_in, **extra})

    global _last_in_maps, _last_mm, _last_build
    _last_in_maps = in_maps
    _last_mm = mm
    _last_build = build
    nc = build(mm)
    res = run_bass_kernel_spmd(nc, in_maps, core_ids=list(range(8)))

    out = np.empty((N, OC, D, H, W), np.float32)
    for ci in range(8):
        n, dc = divmod(ci, 4)
        out[n, :, 4 * dc : 4 * dc + DCHUNK] = res.results[ci]["out"].reshape(
            OC, DCHUNK, H, W
        )
    return out


# revision 31
# speedup vs baseline: 1.2252x; 1.1822x over previous
"""Dcls3d (learnable-position dilated conv3d) Trainium2 kernel.

Reference computes:
  K = trilinear-scatter(weight, P) -> (64, 32, 5, 5, 5)
  out = conv3d(x, K, stride 1, pad 2) + bias     x: (2,32,16,32,32) -> out: (2,64,16,32,32)

Strategy (8 cores): shard (batch n in {0,1}) x (4 chunks of 4 output d-planes).
Each core runs an implicit-GEMM direct conv:
  - input slab (zero-padded on host) replicated 4x in SBUF, w-shifted by
    delta=0..3, giving a 128-partition (delta, ic) contraction axis.
  - for each of 25 (l, j) kernel-tap pairs: one matmul contracting
    (4 w-taps x 32 ic) = 128, M=64 out-channels, N=512 outputs, accumulating
    in PSUM; the i=4 leftover tap runs as a K=32 matmul off the delta-group.
  - bias added during PSUM->SBUF copyback; one 1MB store per core.
"""

import numpy as np

import concourse.bass as bass
import concourse.bacc as bacc
import concourse.mybir as mybir
from concourse.bass_utils import run_bass_kernel_spmd
from concourse.tile import TileContext

# ---- problem constants (hardcoded per contract) ----
N, IC, D, H, W = 2, 32, 16, 32, 32
OC = 64
KC = 16
PAD = 2
DP, HP, WP = D + 2 * PAD, H + 2 * PAD, W + 2 * PAD  # 20, 36, 36
DCHUNK = 4              # output d-planes per core
DSLAB = DCHUNK + 4      # input d-planes per core (halo 2 each side)
SLABF = DSLAB * HP * WP  # 8*36*36 = 10368
XS_COLS = SLABF + 4     # slack so the delta-shifted loads stay in bounds
NTAPS_LJ = 25
OUTF = DCHUNK * H * W   # 4096 outputs per (core, oc)

_NC_CACHE = {}


def _construct_K(weight, P):
    """Exact numpy port of reference.construct_kernel for ks=(5,5,5)."""
    Pp = P + np.float32(2.0)
    Pf = np.floor(Pp)
    R = Pp - Pf
    P1, P2, P3 = Pf[0], Pf[1], Pf[2]
    R1, R2, R3 = R[0], R[1], R[2]
    g = np.arange(5, dtype=P.dtype)[:, None, None, None]
    aL = (g == P1) * (1.0 - R1) + (g == P1 + 1.0) * R1
    aJ = (g == P3) * (1.0 - R3) + (g == P3 + 1.0) * R3
    aI = (g == P2) * (1.0 - R2) + (g == P2 + 1.0) * R2
    K = np.einsum("ock,lock,jock,iock->oclji", weight, aL, aJ, aI, optimize=True)
    return np.ascontiguousarray(K.astype(np.float32))


LJ_A = [lj for lj in range(NTAPS_LJ) if lj % 2 == 0]  # col-group 0 taps
LJ_B = [lj for lj in range(NTAPS_LJ) if lj % 2 == 1]  # col-group 1 taps
ROW_PACK = False  # leftover i=4 taps spread across PE row groups


def _build_nc_packed(mm="bf16"):
    """v1: col-group packed (2 taps concurrently on PE) + row-packed i=4."""
    key = ("v1", mm, ROW_PACK)
    if key in _NC_CACHE:
        return _NC_CACHE[key]
    f32 = mybir.dt.float32
    mdt = {"f32": f32, "bf16": mybir.dt.bfloat16}[mm]
    nc = bacc.Bacc()
    xs = nc.dram_tensor("xs", [IC, XS_COLS], mdt, kind="ExternalInput")
    kta = nc.dram_tensor("kta", [128, len(LJ_A) * OC], mdt, kind="ExternalInput")
    ktb = nc.dram_tensor("ktb", [128, len(LJ_B) * OC], mdt, kind="ExternalInput")
    ktd = nc.dram_tensor("ktd", [128, 5 * OC], mdt, kind="ExternalInput")
    ktj = nc.dram_tensor("ktj", [128, OC], mdt, kind="ExternalInput")
    kt5 = nc.dram_tensor("kt5", [IC, OC], mdt, kind="ExternalInput")
    bias = nc.dram_tensor("bias", [OC, 1], f32, kind="ExternalInput")
    out = nc.dram_tensor("out", [OC, OUTF], f32, kind="ExternalOutput")

    HALF = 6 * HP * WP  # six d-planes per xrep half
    with TileContext(nc) as tc:
        with (
            tc.tile_pool(name="const", bufs=1) as cpool,
            tc.tile_pool(name="psum", bufs=8, space="PSUM") as ppool,
        ):
            kta_sb = cpool.tile([128, len(LJ_A) * OC], mdt)
            nc.sync.dma_start(out=kta_sb, in_=kta[:, :])
            ktb_sb = cpool.tile([128, len(LJ_B) * OC], mdt)
            nc.sync.dma_start(out=ktb_sb, in_=ktb[:, :])
            ktd_sb = cpool.tile([128, 5 * OC], mdt)
            nc.sync.dma_start(out=ktd_sb, in_=ktd[:, :])
            ktj_sb = cpool.tile([128, OC], mdt)
            nc.sync.dma_start(out=ktj_sb, in_=ktj[:, :])
            kt5_sb = cpool.tile([IC, OC], mdt)
            nc.sync.dma_start(out=kt5_sb, in_=kt5[:, :])
            bias_sb = cpool.tile([OC, 1], f32)
            nc.sync.dma_start(out=bias_sb, in_=bias[:, :])
            # input slab split in two halves (planes 0-5 / 2-7) so out d=0,1
            # compute starts while the second half still loads
            xrepA = cpool.tile([128, HALF], mdt)
            xrepB = cpool.tile([128, HALF], mdt)
            for dl in range(4):
                nc.sync.dma_start(
                    out=xrepA[dl * IC : (dl + 1) * IC, :], in_=xs[:, dl : dl + HALF]
                )
            for dl in range(4):
                nc.sync.dma_start(
                    out=xrepB[dl * IC : (dl + 1) * IC, :],
                    in_=xs[:, 2 * HP * WP + dl : 2 * HP * WP + dl + HALF],
                )
            obufs = [cpool.tile([OC, H * W], f32, name=f"obuf{d}") for d in range(4)]

            # d-shifted replication for the i=4 taps: partition group
            # lam holds xs shifted by lam d-planes AND +4 in w, so one
            # K=128 matmul covers taps (l=lam, j, i=4) for lam=0..3.
            DWIN = 4 * HP * WP
            xrepD = cpool.tile([128, DWIN], mdt)
            for lam in range(4):
                o = lam * HP * WP + 4
                nc.sync.dma_start(
                    out=xrepD[lam * IC : (lam + 1) * IC, :], in_=xs[:, o : o + DWIN]
                )
            # h-row (j) shifted replication for taps (l=4, j=0..3, i=4):
            # partition group mu holds planes 4..7 shifted by mu rows and +4 w
            JWIN = 5040
            xrepJ = cpool.tile([128, JWIN], mdt)
            for mu in range(4):
                o = 4 * HP * WP + mu * WP + 4
                nc.sync.dma_start(
                    out=xrepJ[mu * IC : (mu + 1) * IC, :], in_=xs[:, o : o + JWIN]
                )

            xrepA_r = xrepA.rearrange("p (r w) -> p r w", w=WP)
            xrepB_r = xrepB.rearrange("p (r w) -> p r w", w=WP)
            xrepD_r = xrepD.rearrange("p (r w) -> p r w", w=WP)
            xrepJ_r = xrepJ.rearrange("p (r w) -> p r w", w=WP)

            def tile_geom(t):
                d, h0 = divmod(t, 2)
                h0 *= 16
                xr = xrepA_r if d < 2 else xrepB_r
                dbase = 0 if d < 2 else 2
                return d, h0, xr, dbase

            # pass 1: all w-packed taps (need only xrepA/xrepB) for all 8
            # tiles -- 8 psum banks accumulate concurrently, so the PE never
            # stalls on the later xrepD/xrepJ DMAs.
            pss = []
            for t in range(8):
                d, h0, xrep_r, dbase = tile_geom(t)
                ps = ppool.tile([128, 512], f32)
                pss.append(ps)
                for s in range(len(LJ_A)):
                    for grp, ljs, ktsb in ((0, LJ_A, kta_sb), (1, LJ_B, ktb_sb)):
                        if s >= len(ljs):
                            continue
                        lj = ljs[s]
                        l, j = divmod(lj, 5)
                        r = (d + l - dbase) * HP + h0 + j
                        nc.tensor.matmul(
                            ps[grp * 64 : grp * 64 + 64, :],
                            ktsb[:, s * OC : (s + 1) * OC],
                            xrep_r[:, r : r + 16, 0:W],
                            start=(s == 0),
                            stop=False,
                            skip_group_check=True,
                            tile_position=(0, grp * 64),
                        )
            # pass 2: i=4 closers off xrepD/xrepJ + corner single + epilogue
            for t in range(8):
                d, h0, xrep_r, dbase = tile_geom(t)
                ps = pss[t]
                for j in range(5):
                    grp = j % 2
                    nc.tensor.matmul(
                        ps[grp * 64 : grp * 64 + 64, :],
                        ktd_sb[:, j * OC : (j + 1) * OC],
                        xrepD_r[:, d * HP + h0 + j : d * HP + h0 + j + 16, 0:W],
                        start=False,
                        stop=False,
                        skip_group_check=True,
                        tile_position=(0, grp * 64),
                    )
                nc.tensor.matmul(
                    ps[64:128, :],
                    ktj_sb[:, :],
                    xrepJ_r[:, d * HP + h0 : d * HP + h0 + 16, 0:W],
                    start=False,
                    stop=True,
                    skip_group_check=True,
                    tile_position=(0, 64),
                )
                r45 = (d + 4 - dbase) * HP + h0 + 4  # tap (l=4, j=4)
                nc.tensor.matmul(
                    ps[0:64, :],
                    kt5_sb[0:IC, :],
                    xrep_r[0:IC, r45 : r45 + 16, 4 : 4 + W],
                    start=False,
                    stop=True,
                    skip_group_check=True,
                    tile_position=(0, 0),
                )
                oslice = obufs[d][:, (t % 2) * 512 : (t % 2) * 512 + 512]
                nc.vector.tensor_scalar_add(out=oslice, in0=ps[0:64, :], scalar1=bias_sb)
                nc.vector.tensor_tensor(
                    out=oslice, in0=ps[64:128, :], in1=oslice,
                    op=mybir.AluOpType.add,
                )
                if t % 2 == 1:
                    nc.sync.dma_start(
                        out=out[:, d * H * W : (d + 1) * H * W], in_=obufs[d]
                    )
    nc.finalize()
    _NC_CACHE[key] = nc
    return nc


def _build_nc(mm="bf16"):
    key = ("v0", mm)
    if key in _NC_CACHE:
        return _NC_CACHE[key]
    f32 = mybir.dt.float32
    mdt = {"f32": f32, "bf16": mybir.dt.bfloat16}[mm]
    nc = bacc.Bacc()
    xs = nc.dram_tensor("xs", [IC, XS_COLS], mdt, kind="ExternalInput")
    kt = nc.dram_tensor("kt", [128, NTAPS_LJ * OC], mdt, kind="ExternalInput")
    kt4 = nc.dram_tensor("kt4", [IC, NTAPS_LJ * OC], mdt, kind="ExternalInput")
    bias = nc.dram_tensor("bias", [OC, 1], f32, kind="ExternalInput")
    out = nc.dram_tensor("out", [OC, OUTF], f32, kind="ExternalOutput")

    with TileContext(nc) as tc:
        with (
            tc.tile_pool(name="const", bufs=1) as cpool,
            tc.tile_pool(name="psum", bufs=4, space="PSUM") as ppool,
        ):
            xrep = cpool.tile([128, SLABF], mdt)
            # partition p = dl*32+ic holds xs[ic, dl : dl+SLABF] (w-shift by dl)
            for dl in range(4):
                nc.sync.dma_start(
                    out=xrep[dl * IC : (dl + 1) * IC, :], in_=xs[:, dl : dl + SLABF]
                )
            kt_sb = cpool.tile([128, NTAPS_LJ * OC], mdt)
            nc.sync.dma_start(out=kt_sb, in_=kt[:, :])
            kt4_sb = cpool.tile([IC, NTAPS_LJ * OC], mdt)
            nc.sync.dma_start(out=kt4_sb, in_=kt4[:, :])
            bias_sb = cpool.tile([OC, 1], f32)
            nc.sync.dma_start(out=bias_sb, in_=bias[:, :])
            obuf = cpool.tile([OC, OUTF], f32)

            # view xrep free dim as (row, w) where row = d*HP + h
            xrep_r = xrep.rearrange("p (r w) -> p r w", w=WP)

            for t in range(8):  # out tile: 512 outputs = 16 h-rows x 32 w
                d, h0 = divmod(t, 2)
                h0 *= 16
                ps = ppool.tile([OC, 512], f32)
                for lj in range(NTAPS_LJ):
                    l, j = divmod(lj, 5)
                    r = (d + l) * HP + h0 + j
                    rhs = xrep_r[:, r : r + 16, 0:W]
                    nc.tensor.matmul(
                        ps,
                        kt_sb[:, lj * OC : (lj + 1) * OC],
                        rhs,
                        start=(lj == 0),
                        stop=False,
                    )
                    rhs4 = xrep_r[0:IC, r : r + 16, 4 : 4 + W]
                    nc.tensor.matmul(
                        ps,
                        kt4_sb[:, lj * OC : (lj + 1) * OC],
                        rhs4,
                        start=False,
                        stop=(lj == NTAPS_LJ - 1),
                    )
                nc.vector.tensor_scalar_add(
                    out=obuf[:, t * 512 : (t + 1) * 512], in0=ps, scalar1=bias_sb
                )
            nc.sync.dma_start(out=out[:, :], in_=obuf)
    nc.finalize()
    _NC_CACHE[key] = nc
    return nc


def kernel(x, weight, P, bias, mm="bf16", ver="v1"):
    import ml_dtypes

    x = np.ascontiguousarray(np.asarray(x, dtype=np.float32))
    weight = np.asarray(weight, dtype=np.float32)
    P = np.asarray(P, dtype=np.float32)
    bias = np.asarray(bias, dtype=np.float32)
    mnp = {"f32": np.float32, "bf16": ml_dtypes.bfloat16}[mm]

    K = _construct_K(weight, P)  # (oc, ic, l, j, i)
    # lhsT layouts: partition=(i, ic), free=(l*5+j slot, oc)
    Kt = K.transpose(4, 1, 2, 3, 0)  # (i, ic, l, j, oc)
    KtF = Kt.reshape(5, IC, NTAPS_LJ, OC)
    bias_in = np.ascontiguousarray(bias.reshape(OC, 1))

    xpad = np.pad(x, ((0, 0), (0, 0), (PAD, PAD), (PAD, PAD), (PAD, PAD)))

    if ver == "v0":
        kt = np.ascontiguousarray(KtF[:4].reshape(128, NTAPS_LJ * OC).astype(mnp))
        kt4 = np.ascontiguousarray(KtF[4].reshape(IC, NTAPS_LJ * OC).astype(mnp))
        extra = {"kt": kt, "kt4": kt4}
        build = _build_nc
    else:
        kta = np.ascontiguousarray(
            KtF[:4][:, :, LJ_A, :].reshape(128, len(LJ_A) * OC).astype(mnp)
        )
        ktb = np.ascontiguousarray(
            KtF[:4][:, :, LJ_B, :].reshape(128, len(LJ_B) * OC).astype(mnp)
        )
        # ktd: partition (l, ic) for l=0..3, free (j, oc): taps (l, j, i=4)
        ktd = np.zeros((128, 5 * OC), mnp)
        for j in range(5):
            for l in range(4):
                ktd[32 * l : 32 * (l + 1), j * OC : (j + 1) * OC] = KtF[
                    4, :, l * 5 + j, :
                ].astype(mnp)
        # ktj: partition (j, ic) for j=0..3: taps (l=4, j, i=4)
        ktj = np.zeros((128, OC), mnp)
        for j in range(4):
            ktj[32 * j : 32 * (j + 1), :] = KtF[4, :, 4 * 5 + j, :].astype(mnp)
        kt5 = np.ascontiguousarray(KtF[4, :, 24, :].astype(mnp))  # (l=4,j=4,i=4)
        extra = {"kta": kta, "ktb": ktb, "ktd": ktd, "ktj": ktj, "kt5": kt5}
        build = _build_nc_packed

    in_maps = []
    for ci in range(8):
        n, dc = divmod(ci, 4)
        slab = xpad[n, :, 4 * dc : 4 * dc + DSLAB].reshape(IC, SLABF)
        xs = np.zeros((IC, XS_COLS), mnp)
        xs[:, :SLABF] = slab.astype(mnp)
        in_maps.append({"xs": xs, "bias": bias_in, **extra})

    global _last_in_maps, _last_mm, _last_build
    _last_in_maps = in_maps
    _last_mm = mm
    _last_build = build
    nc = build(mm)
    res = run_bass_kernel_spmd(nc, in_maps, core_ids=list(range(8)))

    out = np.empty((N, OC, D, H, W), np.float32)
    for ci in range(8):
        n, dc = divmod(ci, 4)
        out[n, :, 4 * dc : 4 * dc + DCHUNK] = res.results[ci]["out"].reshape(
            OC, DCHUNK, H, W
        )
    return out
